# revision 4
# baseline (speedup 1.0000x reference)
"""GAT (3-layer, 4-head) forward on 8 Trainium2 NeuronCores — v2.

This stack has a huge per-instruction dispatch cost (~13-50us measured), so
the design minimizes instruction count: edge phase processes CHUNKS of
consecutive dst blocks with a uniform padded round count, so softmax,
weighting, and the per-node reduction are a handful of big strided DVE ops
per chunk instead of per-round matmuls. Attention logits are recomputed
on-chip from the gathered bf16 rows (rows are plain 512B h vectors).
Next-layer transposed activations come from two xbar DMA-transposes.

Node partitioning / degree sort / dst-routed slot layout as v1.
"""
import sys
sys.path.insert(0, "/opt/trn_rl_repo")
import os
import numpy as np

# ---- problem constants ----
N = 50000
E = 800000
D = 128
H = 4
C = 64
HC = 256
B = 64
OUT = 10
SLOPE = 0.2

NCOR = 8
NLOC = N // NCOR          # 6250
PBLK = 128
NBLK = (NLOC + PBLK - 1) // PBLK      # 49
NLOCP = NBLK * PBLK       # 6272
NTAB = NCOR * NLOCP       # 50176
RB = 256                  # bf16 row: 256 h values = 512B
SPLIT = 32768             # int16 gather index limit
WA_END = 32768            # window A: table rows [0, 32768)
WB_START = NCOR * ((N // NCOR + 127) // 128 * 128) - 32768   # 17408
F32 = np.float32
GCH = 8                   # rounds per gather instruction (8*128 = 1024 idx)


def _smax():
    return int(os.environ.get("GAT_SMAX", "104"))


# ======================================================================
# host-side preprocessing
# ======================================================================

def _wrap16(idx_flat):
    n = idx_flat.size
    assert n % 16 == 0
    a = idx_flat.reshape(n // 16, 16).T.astype(np.int16)
    return np.tile(a, (8, 1))                              # [128, n/16]


def make_chunks(TLO, THI, smax):
    """DP partition of consecutive blocks into chunks with uniform padded
    rounds (nb*(maxTLO+maxTHI) <= smax), minimizing total padded slots with
    a small per-chunk instruction penalty."""
    INF = 1 << 40
    CHUNK_PENALTY = 12   # rounds-equivalent cost of one extra chunk
    best = [INF] * (NBLK + 1)
    prev = [0] * (NBLK + 1)
    best[0] = 0
    for e in range(1, NBLK + 1):
        tlo = 0; thi = 0
        for b0 in range(e - 1, -1, -1):
            tlo = max(tlo, int(TLO[b0])); thi = max(thi, int(THI[b0]))
            nb = e - b0
            if nb * (tlo + thi) > smax:
                break
            cost = best[b0] + nb * (tlo + thi) + CHUNK_PENALTY
            if cost < best[e]:
                best[e] = cost
                prev[e] = b0
    chunks = []
    e = NBLK
    while e > 0:
        b0 = prev[e]
        tlo = int(max(TLO[b0:e])); thi = int(max(THI[b0:e]))
        chunks.append((b0, e - b0, tlo, thi))
        e = b0
    return chunks[::-1]


def preprocess(x, edge_index, batch, smax):
    src0 = edge_index[0].astype(np.int64)
    dst0 = edge_index[1].astype(np.int64)
    loop = np.arange(N, dtype=np.int64)
    src = np.concatenate([src0, loop])
    dst = np.concatenate([dst0, loop])

    deg = np.bincount(dst, minlength=N)

    # --- node id assignment: iterate (deg, mustA)-sort so blocks have
    # homogeneous window loads; windows A=[0,32768), B=[17408,50176) overlap,
    # edges with src in the overlap can use either (flex) ---
    newid = np.empty(N, dtype=np.int64)
    for c in range(NCOR):
        lo, hi = c * NLOC, (c + 1) * NLOC
        order = np.argsort(deg[lo:hi], kind="stable")
        newid[lo + order] = c * NLOCP + np.arange(NLOC)
    for _ in range(2):
        src_n = newid[src]
        mustA_n = np.bincount(dst, weights=(src_n < WB_START).astype(np.float64),
                              minlength=N).astype(np.int64)
        newid2 = np.empty(N, dtype=np.int64)
        for c in range(NCOR):
            lo, hi = c * NLOC, (c + 1) * NLOC
            keys = deg[lo:hi] * 10000 + mustA_n[lo:hi]
            order = np.argsort(keys, kind="stable")
            newid2[lo + order] = c * NLOCP + np.arange(NLOC)
        newid = newid2
    perm_per_core = []
    inv = np.argsort(newid)           # old ids ordered by new id (rank order)
    for c in range(NCOR):
        # core c's new ids are exactly c*NLOCP + [0, NLOC) -> ranks
        # [c*NLOC, (c+1)*NLOC) in the sorted order
        perm_per_core.append(inv[c * NLOC:(c + 1) * NLOC])

    src_n = newid[src]
    dst_n = newid[dst]
    core_of = dst_n // NLOCP
    dloc = dst_n % NLOCP
    blk = dloc // PBLK
    p = dloc % PBLK
    cat = np.where(src_n < WB_START, 0, np.where(src_n < WA_END, 1, 2))

    # per (core, blk, p) category counts
    key3 = ((core_of * NBLK + blk) * PBLK + p) * 3 + cat
    cnt3 = np.bincount(key3, minlength=NCOR * NBLK * PBLK * 3).reshape(
        NCOR, NBLK, PBLK, 3)
    mA = cnt3[:, :, :, 0]
    fx = cnt3[:, :, :, 1]
    mB = cnt3[:, :, :, 2]

    # per block (pooled across cores): exact min TA + TB(TA)
    TA = np.zeros(NBLK, dtype=np.int64)
    TB = np.zeros(NBLK, dtype=np.int64)
    for b in range(NBLK):
        mAb = mA[:, b, :].ravel(); mBb = mB[:, b, :].ravel()
        fxb = fx[:, b, :].ravel()
        best = None
        for ta in range(int(mAb.max()), int((mAb + fxb).max()) + 1):
            room = ta - mAb
            tb = int((mBb + np.maximum(fxb - room, 0)).max())
            if best is None or ta + tb < best[0] + best[1]:
                best = (ta, tb)
        TA[b], TB[b] = best

    # flex going to window A per (core, blk, p)
    aab = np.minimum(TA[None, :, None] - mA, fx)
    nA = mA + aab               # window-A count per (c,b,p)
    nB = mB + fx - aab          # window-B count

    chunks = make_chunks(TA, TB, smax)
    roff_blk = np.zeros(NBLK, dtype=np.int64)
    chunk_meta = []
    roff = 0
    for (b0, nb, tlo, thi) in chunks:
        chunk_meta.append((b0, nb, tlo, thi, roff))
        for bb in range(nb):
            roff_blk[b0 + bb] = roff + bb * (tlo + thi)
        roff += nb * (tlo + thi)
    TOTP = roff
    tlo_of_blk = np.zeros(NBLK, dtype=np.int64)
    thi_of_blk = np.zeros(NBLK, dtype=np.int64)
    for (b0, nb, tlo, thi, ro) in chunk_meta:
        tlo_of_blk[b0:b0 + nb] = tlo
        thi_of_blk[b0:b0 + nb] = thi

    # --- per-edge slot assignment ---
    key = ((core_of * NBLK + blk) * PBLK + p) * 3 + cat
    order = np.argsort(key, kind="stable")
    key_s = key[order]
    src_s = src_n[order]
    grp_start = np.r_[0, np.flatnonzero(np.diff(key_s)) + 1]
    grp_id = np.zeros(key_s.size, dtype=np.int64)
    grp_id[grp_start[1:]] = 1
    grp_id = np.cumsum(grp_id)
    rank = np.arange(key_s.size) - grp_start[grp_id]

    cbp = key_s // 3
    cat_s = key_s % 3
    core_s = cbp // (NBLK * PBLK)
    blk_s = (cbp // PBLK) % NBLK
    p_s = cbp % PBLK
    mA_e = mA[core_s, blk_s, p_s]
    mB_e = mB[core_s, blk_s, p_s]
    aab_e = aab[core_s, blk_s, p_s]
    TA_e = TA[blk_s]

    # window + round within (c,b,p)
    isA = np.where(cat_s == 0, True, np.where(cat_s == 2, False, rank < aab_e))
    rnd = np.where(cat_s == 0, rank,
                   np.where(cat_s == 2, TA_e + rank,
                            np.where(rank < aab_e, mA_e + rank,
                                     TA_e + mB_e + rank - aab_e)))
    idxval = np.where(isA, src_s, src_s - WB_START).astype(np.int16)

    idx_all = np.zeros((NCOR, PBLK, 8 * TOTP), dtype=np.int16)
    maskmul = np.zeros((NCOR, PBLK, TOTP), dtype=F32)

    for c in range(NCOR):
        for b in range(NBLK):
            ro = roff_blk[b]
            tlo = int(tlo_of_blk[b]); thi = int(thi_of_blk[b])
            t_lo = np.arange(tlo)
            maskmul[c, :, ro:ro + tlo] = (t_lo[None, :] < nA[c, b][:, None])
            t_hi = np.arange(thi)
            maskmul[c, :, ro + tlo:ro + tlo + thi] = (
                t_hi[None, :] < nB[c, b][:, None])

    # idx fill: per (core, blk): A rounds [0,tlo), B rounds [tlo, tlo+thi)
    # edge slot inside block: A: rnd ; B: tlo + (rnd - TA[b])
    slot_t = np.where(isA, rnd, tlo_of_blk[blk_s] + rnd - TA_e)
    for c in range(NCOR):
        mc = core_s == c
        for b in range(NBLK):
            ro = roff_blk[b]
            T_pad = int(tlo_of_blk[b] + thi_of_blk[b])
            m = mc & (blk_s == b)
            flat = np.zeros(T_pad * PBLK, dtype=np.int16)
            flat[slot_t[m] * PBLK + p_s[m]] = idxval[m]
            idx_all[c, :, ro * 8:(ro + T_pad) * 8] = _wrap16(flat)

    # batch one-hot with 1/(4*cnt) folded (head mean + graph mean)
    counts = np.bincount(batch.astype(np.int64), minlength=B).astype(F32)
    counts = np.maximum(counts, 1.0)
    import ml_dtypes
    bh_all = np.zeros((NCOR, PBLK, NBLK * B), dtype=ml_dtypes.bfloat16)
    xT0 = np.zeros((NCOR, D, NLOCP), dtype=F32)
    for c in range(NCOR):
        old = perm_per_core[c]
        bt = batch[old].astype(np.int64)          # [NLOC]
        onehot = np.zeros((NLOCP, B), dtype=F32)
        onehot[np.arange(NLOC), bt] = 0.25 / counts[bt]
        # node (b, p) -> row b*128+p ; want [128, NBLK, B]
        bh = onehot.reshape(NBLK, PBLK, B).transpose(1, 0, 2).reshape(
            PBLK, NBLK * B)
        bh_all[c] = bh.astype(ml_dtypes.bfloat16)
        xT0[c, :, :NLOC] = x[old].T

    static = dict(chunks=chunk_meta, TOTP=TOTP,
                  tlo_of_blk=tlo_of_blk, thi_of_blk=thi_of_blk)
    percore = dict(idx_all=idx_all, maskmul=maskmul, bh_all=bh_all, xT0=xT0)
    return static, percore


# ======================================================================
# bass program
# ======================================================================

def build_program(static):
    import concourse.bacc as bacc
    import concourse.mybir as mybir
    import concourse.tile as tile
    from concourse.library_config import mlp

    f32 = mybir.dt.float32
    bf16 = mybir.dt.bfloat16
    AFT = mybir.ActivationFunctionType
    ALU = mybir.AluOpType
    chunks = static["chunks"]
    TOTP = static["TOTP"]

    gch = int(os.environ.get("GAT_GCH", str(GCH)))
    gbufs = int(os.environ.get("GAT_GBUFS", "1"))
    nqueues = int(os.environ.get("GAT_QUEUES", "1"))
    n_layers = int(os.environ.get("GAT_LAYERS", "3"))
    n_rep = int(os.environ.get("GAT_REPEAT", "1"))
    no_cc = os.environ.get("GAT_NOCC", "0") == "1"
    lvl = int(os.environ.get("GAT_EDGEOPS", "9"))
    nchunk_cap = int(os.environ.get("GAT_CHUNKCAP", str(len(chunks))))

    nc = bacc.Bacc(None, target_bir_lowering=False, num_devices=NCOR,
                   num_swdge_queues=nqueues)

    # ---- I/O ----
    xT0_d = nc.dram_tensor("xT0", [D, NLOCP], f32, kind="ExternalInput")
    W0_d = nc.dram_tensor("W0", [D, HC], f32, kind="ExternalInput")
    W1_d = nc.dram_tensor("W1", [HC, HC], bf16, kind="ExternalInput")
    W2_d = nc.dram_tensor("W2", [HC, HC], bf16, kind="ExternalInput")
    asr_d = [nc.dram_tensor(f"asrep{l}", [PBLK, HC], bf16, kind="ExternalInput")
             for l in range(3)]
    adr_d = [nc.dram_tensor(f"adrep{l}", [PBLK, HC], bf16, kind="ExternalInput")
             for l in range(3)]
    brep_d = {0: nc.dram_tensor("b0rep", [PBLK, HC], f32, kind="ExternalInput"),
              1: nc.dram_tensor("b1rep", [PBLK, HC], f32, kind="ExternalInput")}
    b2c_d = nc.dram_tensor("b2col", [C, 1], f32, kind="ExternalInput")
    idx_d = nc.dram_tensor("idx_all", [PBLK, 8 * TOTP], mybir.dt.int16,
                           kind="ExternalInput")
    msk_d = nc.dram_tensor("maskmul", [PBLK, TOTP], f32, kind="ExternalInput")
    bh_d = nc.dram_tensor("bh_all", [PBLK, NBLK * B], bf16, kind="ExternalInput")
    pw1_d = nc.dram_tensor("pW1", [C, C // 2], f32, kind="ExternalInput")
    pb1_d = nc.dram_tensor("pb1", [C // 2, 1], f32, kind="ExternalInput")
    pw2_d = nc.dram_tensor("pW2", [C // 2, OUT], f32, kind="ExternalInput")
    pb2_d = nc.dram_tensor("pb2", [OUT, 1], f32, kind="ExternalInput")
    out_d = nc.dram_tensor("out_t", [OUT, B], f32, kind="ExternalOutput")

    # ---- internals ----
    h_loc = [nc.dram_tensor(f"h_loc{l}", [NLOCP, RB], bf16) for l in range(3)]
    ag = [nc.dram_tensor(f"ag{l}", [NTAB, RB], bf16, addr_space="Shared")
          for l in range(3)]
    xnew = [nc.dram_tensor(f"xnew{l}", [NLOCP, HC], bf16) for l in range(2)]
    pool_in = nc.dram_tensor("pool_in", [C, B], f32)
    pool_out = nc.dram_tensor("pool_out", [C, B], f32, addr_space="Shared")

    groups = [list(range(NCOR))]

    with tile.TileContext(nc) as tc:
        with tc.tile_pool(name="const", bufs=1) as cp, \
             tc.tile_pool(name="state", bufs=1) as st, \
             tc.tile_pool(name="dense", bufs=3) as dp, \
             tc.tile_pool(name="work", bufs=2) as wp, \
             tc.tile_pool(name="wq", bufs=1) as wq, \
             tc.tile_pool(name="fin", bufs=1) as fp, \
             tc.tile_pool(name="gbuf", bufs=gbufs) as gp, \
             tc.tile_pool(name="psum_d", bufs=2, space="PSUM") as pd, \
             tc.tile_pool(name="psum_t", bufs=2, space="PSUM") as pt, \
             tc.tile_pool(name="psum_g", bufs=1, space="PSUM") as pg:

            nc.gpsimd.load_library(mlp)

            # ---- constants ----
            W0sb = cp.tile([PBLK, HC], dtype=f32)
            nc.sync.dma_start(out=W0sb[:], in_=W0_d[:])
            Wsb = {}
            for ell, wd in ((1, W1_d), (2, W2_d)):
                for cc in range(2):
                    w = cp.tile([PBLK, HC], dtype=bf16, name=f"W{ell}c{cc}")
                    nc.sync.dma_start(out=w[:], in_=wd[cc * PBLK:(cc + 1) * PBLK, :])
                    Wsb[(ell, cc)] = w
            asr = []
            adr = []
            for l in range(3):
                a = cp.tile([PBLK, HC], dtype=bf16, name=f"asr{l}")
                nc.sync.dma_start(out=a[:], in_=asr_d[l][:])
                asr.append(a)
                a2 = cp.tile([PBLK, HC], dtype=bf16, name=f"adr{l}")
                nc.sync.dma_start(out=a2[:], in_=adr_d[l][:])
                adr.append(a2)
            brep = {}
            for l in range(2):
                bb_ = cp.tile([PBLK, HC], dtype=f32, name=f"brep{l}")
                nc.sync.dma_start(out=bb_[:], in_=brep_d[l][:])
                brep[l] = bb_
            bh_sb = cp.tile([PBLK, NBLK * B], dtype=bf16)
            nc.sync.dma_start(out=bh_sb[:], in_=bh_d[:])
            b2c = cp.tile([C, 1], dtype=f32)
            nc.sync.dma_start(out=b2c[:], in_=b2c_d[:])
            pw1 = cp.tile([C, C // 2], dtype=f32)
            nc.sync.dma_start(out=pw1[:], in_=pw1_d[:])
            pb1 = cp.tile([C // 2, 1], dtype=f32)
            nc.sync.dma_start(out=pb1[:], in_=pb1_d[:])
            pw2 = cp.tile([C // 2, OUT], dtype=f32)
            nc.sync.dma_start(out=pw2[:], in_=pw2_d[:])
            pb2 = cp.tile([OUT, 1], dtype=f32)
            nc.sync.dma_start(out=pb2[:], in_=pb2_d[:])

            # ---- state tiles (reused across layers) ----
            xT_sb = [st.tile([PBLK, NLOCP], dtype=bf16, name=f"xTsb{cc}")
                     for cc in range(2)]
            if lvl < 6:   # ablation modes never write xT_sb; keep reads legal
                for cc in range(2):
                    nc.vector.memset(xT_sb[cc][:], 0.0)
            hsb_all = st.tile([PBLK, NBLK * HC], dtype=bf16)
            OUT_all = st.tile([PBLK, NBLK * HC], dtype=bf16)
            den_all = st.tile([PBLK, NBLK * H], dtype=f32)
            ald_blk = st.tile([PBLK, NBLK * H], dtype=f32)

            pool_ps = pg.tile([C, B], dtype=f32, space="PSUM")

            for rep, ell in [(r, l) for r in range(n_rep)
                             for l in range(n_layers)]:
                # ======== dense phase: h = x @ W ========
                for b in range(NBLK):
                    ph = pd.tile([PBLK, HC], dtype=f32, space="PSUM", tag="ph")
                    if ell == 0:
                        lw = dp.tile([PBLK, PBLK], dtype=f32, tag="lw")
                        nc.sync.dma_start(
                            out=lw[:], in_=xT0_d[:, b * PBLK:(b + 1) * PBLK])
                        nc.tensor.matmul(ph[:], lhsT=lw[:], rhs=W0sb[:],
                                         start=True, stop=True)
                    else:
                        for cc in range(2):
                            nc.tensor.matmul(
                                ph[:],
                                lhsT=xT_sb[cc][:, b * PBLK:(b + 1) * PBLK],
                                rhs=Wsb[(ell, cc)][:],
                                start=(cc == 0), stop=(cc == 1))
                    nc.scalar.activation(
                        hsb_all[:, b * HC:(b + 1) * HC], ph[:], AFT.Copy)

                # ald_blk[p, b, h] = sum_c h * a_dst
                tmpald = fp.tile([PBLK, NBLK * HC], dtype=bf16, tag="fin",
                                 name="tmpald")
                nc.vector.tensor_tensor(
                    out=tmpald[:].rearrange("p (b f) -> p b f", b=NBLK),
                    in0=hsb_all[:].rearrange("p (b f) -> p b f", b=NBLK),
                    in1=adr[ell][:, None, :].to_broadcast([PBLK, NBLK, HC]),
                    op=ALU.mult)
                nc.vector.tensor_reduce(
                    out=ald_blk[:],
                    in_=tmpald[:].rearrange("p (b h c) -> p (b h) c", h=H, c=C),
                    axis=mybir.AxisListType.X, op=ALU.add)

                # h_loc write + halo exchange
                nc.sync.dma_start(
                    out=h_loc[ell][:].rearrange("(b p) c -> p b c", p=PBLK),
                    in_=hsb_all[:].rearrange("p (b c) -> p b c", b=NBLK))
                if no_cc:
                    nc.sync.dma_start(out=ag[ell][0:NLOCP, :], in_=h_loc[ell][:])
                else:
                    nc.gpsimd.collective_compute(
                        "AllGather", mybir.AluOpType.bypass,
                        replica_groups=groups,
                        ins=[h_loc[ell][:]], outs=[ag[ell][:]])

                # ======== edge phase: chunks of blocks ========
                for ci, (b0, nb, tlo, thi, roff) in enumerate(chunks):
                    if ci >= nchunk_cap:
                        break
                    T = tlo + thi
                    S = nb * T
                    G = gp.tile([PBLK, S, RB], dtype=bf16, tag="G",
                                name=f"G{ci}")
                    idxc = wp.tile([PBLK, 8 * S], dtype=mybir.dt.int16,
                                   tag="idxc", name=f"idxc{ci}")
                    nc.sync.dma_start(
                        out=idxc[:], in_=idx_d[:, 8 * roff:8 * (roff + S)])
                    mskc = wp.tile([PBLK, S], dtype=f32, tag="mskc",
                                   name=f"mskc{ci}")
                    nc.sync.dma_start(
                        out=mskc[:], in_=msk_d[:, roff:roff + S])
                    # gathers: per (block, half), <=GCH rounds each
                    gq = 0
                    for bb in range(nb):
                        rbase = bb * T
                        for (hbase, hlen, srcbase) in ((rbase, tlo, 0),
                                                       (rbase + tlo, thi, 1)):
                            src_view = (ag[ell][0:WA_END, :] if srcbase == 0
                                        else ag[ell][WB_START:, :])
                            for r0 in range(0, hlen, gch):
                                r1 = min(r0 + gch, hlen)
                                a0, a1 = hbase + r0, hbase + r1
                                nidx = (r1 - r0) * PBLK
                                nc.gpsimd.dma_gather(
                                    G[:, a0:a1, :], src_view,
                                    idxc[:, a0 * 8:a1 * 8],
                                    nidx, nidx, RB,
                                    queue_num=gq % nqueues)
                                gq += 1
                    if lvl < 2:
                        continue
                    # als (=X) [128, H, S] f32 (head-major): per-head mult+reduce
                    X = wp.tile([PBLK, H, S], dtype=f32, tag="X",
                                name=f"X{ci}")
                    tmph = fp.tile([PBLK, S, 2, C], dtype=bf16, tag="fin",
                                   name=f"tmph{ci}")
                    for hh in (0, 2):
                        nc.vector.tensor_tensor(
                            out=tmph[:],
                            in0=G[:, :, hh * C:(hh + 2) * C]
                                .rearrange("p s (h c) -> p s h c", h=2),
                            in1=asr[ell][:, None, hh * C:(hh + 2) * C]
                                .to_broadcast([PBLK, S, 2 * C])
                                .rearrange("p s (h c) -> p s h c", h=2),
                            op=ALU.mult)
                        nc.vector.tensor_reduce(
                            out=X[:, hh:hh + 2, :]
                                .rearrange("p h s -> p s h"),
                            in_=tmph[:],
                            axis=mybir.AxisListType.X, op=ALU.add)
                    # X += ald (broadcast over rounds)
                    nc.vector.tensor_tensor(
                        out=X[:].rearrange("p h (nb t) -> p h nb t", nb=nb),
                        in0=X[:].rearrange("p h (nb t) -> p h nb t", nb=nb),
                        in1=ald_blk[:].rearrange("p (b h) -> p h b", b=NBLK)
                            [:, :, b0:b0 + nb, None]
                            .to_broadcast([PBLK, H, nb, T]),
                        op=ALU.add)
                    # leaky relu + exp + mask (-> bf16 alpha)
                    Xs = wp.tile([PBLK, H, S], dtype=f32, tag="Xs",
                                 name=f"Xs{ci}")
                    nc.vector.tensor_scalar(out=Xs[:], in0=X[:],
                                            scalar1=SLOPE, scalar2=None,
                                            op0=ALU.mult)
                    nc.vector.tensor_tensor(out=X[:], in0=X[:], in1=Xs[:],
                                            op=ALU.max)
                    nc.scalar.activation(X[:], X[:], AFT.Exp)
                    Xb = wp.tile([PBLK, H, S], dtype=bf16, tag="Xb",
                                 name=f"Xb{ci}")
                    nc.vector.tensor_tensor(
                        out=Xb[:], in0=X[:],
                        in1=mskc[:, None, :].to_broadcast([PBLK, H, S]),
                        op=ALU.mult)
                    if lvl < 3:
                        continue
                    # den[p, h, b] = sum_t alpha
                    nc.vector.tensor_reduce(
                        out=den_all[:].rearrange("p (b h) -> p h b", b=NBLK)
                            [:, :, b0:b0 + nb],
                        in_=Xb[:].rearrange("p h (nb t) -> p h nb t", nb=nb),
                        axis=mybir.AxisListType.X, op=ALU.add)
                    if lvl < 4:
                        continue
                    # G *= alpha (per head)
                    nc.vector.tensor_tensor(
                        out=G[:].rearrange("p s (h c) -> p s h c", h=H),
                        in0=G[:].rearrange("p s (h c) -> p s h c", h=H),
                        in1=Xb[:].rearrange("p h s -> p s h")[:, :, :, None]
                            .to_broadcast([PBLK, S, H, C]),
                        op=ALU.mult)
                    if lvl < 5:
                        continue
                    # OUT[p, b, f] = sum_t G
                    stage = wq.tile([PBLK, nb * HC], dtype=f32, tag="stage",
                                    name=f"stage{ci}")
                    nc.vector.tensor_reduce(
                        out=stage[:].rearrange("p (nb f) -> p nb f", nb=nb),
                        in_=G[:].rearrange("p (nb t) f -> p nb f t", nb=nb),
                        axis=mybir.AxisListType.X, op=ALU.add)
                    nc.scalar.activation(
                        OUT_all[:, b0 * HC:(b0 + nb) * HC], stage[:], AFT.Copy)

                if lvl < 6:
                    continue
                # ======== finale ========
                nc.vector.tensor_scalar(out=den_all[:], in0=den_all[:],
                                        scalar1=1e-16, scalar2=None,
                                        op0=ALU.add)
                nc.vector.reciprocal(den_all[:], den_all[:])
                if ell < 2:
                    XN = hsb_all   # h rows fully consumed; reuse as x^{l+1} buffer
                    nc.vector.tensor_tensor(
                        out=XN[:].rearrange("p (bh c) -> p bh c", c=C),
                        in0=OUT_all[:].rearrange("p (bh c) -> p bh c", c=C),
                        in1=den_all[:][:, :, None]
                            .to_broadcast([PBLK, NBLK * H, C]),
                        op=ALU.mult)
                    nc.vector.tensor_tensor(
                        out=XN[:].rearrange("p (b f) -> p b f", b=NBLK),
                        in0=XN[:].rearrange("p (b f) -> p b f", b=NBLK),
                        in1=brep[ell][:, None, :].to_broadcast([PBLK, NBLK, HC]),
                        op=ALU.add)
                    # ELU
                    mn = fp.tile([PBLK, NBLK * HC], dtype=bf16, tag="fin",
                                 name=f"mn{ell}{rep}")
                    nc.vector.tensor_scalar(out=mn[:], in0=XN[:], scalar1=0.0,
                                            scalar2=None, op0=ALU.min)
                    nc.scalar.activation(mn[:], mn[:], AFT.Exp)
                    nc.vector.tensor_scalar(out=mn[:], in0=mn[:], scalar1=-1.0,
                                            scalar2=None, op0=ALU.add)
                    nc.vector.tensor_tensor(out=XN[:], in0=XN[:], in1=mn[:],
                                            op=ALU.max)
                    # write + transpose for next dense
                    nc.sync.dma_start(
                        out=xnew[ell][:].rearrange("(b p) c -> p b c", p=PBLK),
                        in_=XN[:].rearrange("p (b c) -> p b c", b=NBLK))
                    for cc in range(2):
                        nc.sync.dma_start_transpose(
                            out=xT_sb[cc][:],
                            in_=xnew[ell][:, cc * PBLK:(cc + 1) * PBLK])
                else:
                    # normalize in place + head mean -> o64 [128, NBLK*C]
                    nc.vector.tensor_tensor(
                        out=OUT_all[:].rearrange("p (bh c) -> p bh c", c=C),
                        in0=OUT_all[:].rearrange("p (bh c) -> p bh c", c=C),
                        in1=den_all[:][:, :, None]
                            .to_broadcast([PBLK, NBLK * H, C]),
                        op=ALU.mult)
                    o64 = fp.tile([PBLK, NBLK * C], dtype=f32, tag="fin",
                                  name=f"o64{rep}")
                    nc.vector.tensor_reduce(
                        out=o64[:].rearrange("p (nb c) -> p nb c", c=C),
                        in_=OUT_all[:].rearrange("p (nb h c) -> p nb c h",
                                                 h=H, c=C),
                        axis=mybir.AxisListType.X, op=ALU.add)
                    o64b = wq.tile([PBLK, NBLK * C], dtype=bf16, tag="o64b",
                                   name=f"o64b{rep}")
                    nc.scalar.activation(o64b[:], o64[:], AFT.Copy)
                    for b in range(NBLK):
                        nc.tensor.matmul(
                            pool_ps[:],
                            lhsT=o64b[:, b * C:(b + 1) * C],
                            rhs=bh_sb[:, b * B:(b + 1) * B],
                            start=(b == 0), stop=(b == NBLK - 1))

            # ---- pooled AllReduce + bias + MLP ----
            pool_sb = wp.tile([C, B], dtype=f32, tag="pool_sb")
            if n_layers == 3 and lvl >= 6:
                nc.scalar.activation(pool_sb[:], pool_ps[:], AFT.Copy)
            else:
                nc.vector.memset(pool_sb[:], 0.0)
            nc.sync.dma_start(out=pool_in[:], in_=pool_sb[:])
            if no_cc:
                nc.sync.dma_start(out=pool_out[:], in_=pool_in[:])
            else:
                nc.gpsimd.collective_compute(
                    "AllReduce", mybir.AluOpType.add, replica_groups=groups,
                    ins=[pool_in[:]], outs=[pool_out[:]])
            pooled = wp.tile([C, B], dtype=f32, tag="pooled")
            nc.sync.dma_start(out=pooled[:], in_=pool_out[:])
            nc.vector.tensor_scalar(out=pooled[:], in0=pooled[:],
                                    scalar1=b2c[:, 0:1], scalar2=None,
                                    op0=ALU.add)

            z1p = pt.tile([C // 2, B], dtype=f32, space="PSUM", tag="tp")
            nc.tensor.matmul(z1p[:], lhsT=pw1[:], rhs=pooled[:],
                             start=True, stop=True)
            z1 = wp.tile([C // 2, B], dtype=f32, tag="z1")
            nc.scalar.activation(z1[:], z1p[:], AFT.Relu, bias=pb1[:, 0:1])
            z2p = pt.tile([OUT, B], dtype=f32, space="PSUM", tag="tp")
            nc.tensor.matmul(z2p[:], lhsT=pw2[:], rhs=z1[:],
                             start=True, stop=True)
            z2 = wp.tile([OUT, B], dtype=f32, tag="z2")
            nc.vector.tensor_scalar(out=z2[:], in0=z2p[:], scalar1=pb2[:, 0:1],
                                    scalar2=None, op0=ALU.add)
            nc.sync.dma_start(out=out_d[:], in_=z2[:])

    nc.compile()
    return nc


# ======================================================================
# entry point
# ======================================================================

def prepare(x, edge_index, batch, W0, b0, as0, ad0, W1, b1, as1, ad1,
            W2, b2, as2, ad2, pW1, pb1, pW2, pb2):
    import ml_dtypes
    bfl = ml_dtypes.bfloat16
    x = np.asarray(x, dtype=F32)
    edge_index = np.asarray(edge_index)
    batch = np.asarray(batch)

    static, percore = preprocess(x, edge_index, batch, _smax())
    nc = build_program(static)

    def rep(a):
        return np.broadcast_to(np.asarray(a, F32).reshape(-1), (PBLK, HC)) \
                 .astype(bfl).copy()

    in_common = dict(
        W0=np.asarray(W0, F32),
        W1=np.asarray(W1, F32).astype(bfl),
        W2=np.asarray(W2, F32).astype(bfl),
        asrep0=rep(as0), asrep1=rep(as1), asrep2=rep(as2),
        adrep0=rep(ad0), adrep1=rep(ad1), adrep2=rep(ad2),
        b0rep=np.broadcast_to(np.asarray(b0, F32), (PBLK, HC)).copy(),
        b1rep=np.broadcast_to(np.asarray(b1, F32), (PBLK, HC)).copy(),
        b2col=np.asarray(b2, F32).reshape(C, 1),
        pW1=np.asarray(pW1, F32), pb1=np.asarray(pb1, F32).reshape(-1, 1),
        pW2=np.asarray(pW2, F32), pb2=np.asarray(pb2, F32).reshape(-1, 1),
    )
    in_maps = []
    for c in range(NCOR):
        m = dict(in_common)
        m.update(
            xT0=percore["xT0"][c],
            idx_all=percore["idx_all"][c],
            maskmul=percore["maskmul"][c],
            bh_all=percore["bh_all"][c],
        )
        in_maps.append(m)
    return nc, in_maps


def kernel(**inputs):
    nc, in_maps = prepare(**inputs)
    from concourse.bass_utils import run_bass_kernel_spmd
    res = run_bass_kernel_spmd(nc, in_maps, list(range(NCOR)))
    out_t = res.results[0]["out_t"]            # [OUT, B]
    return np.ascontiguousarray(out_t.T).astype(F32)
